# revision 1
# baseline (speedup 1.0000x reference)
# Trainium2 Bass kernel for nn_FDM_3899830304921 (feature-map cosine-sim
# dual-softmax transport), data-parallel over the batch dim on 8 NeuronCores.
#
# Math per batch (c=512, n=m=784):
#   G[n,m]   = f1^T @ f2                     (gram, contract c)
#   M[n,m]   = -G * (1/||f1_:n||) * (1/||f2_:m||)   (= -cos_sim)
#   E        = exp(M)            (M in [-1,1] -> no max-subtraction needed)
#   new_fm2  = 0.001 * (f1 @ E) / colsum(E)      (softmax over n folded in)
#   new_fm1  = 0.001 * (f2 @ (E*1/rowsum)^T)     (softmax over m folded in)
#
# All matmuls run in float32r (fp32 read, FP22 multiply, fp32 accumulate).
# Every SBUF tile consumed by an fp32r matmul is declared float32r so its
# producer rounds on write (BIR verifier requirement).
import sys

if "/opt/trn_rl_repo" not in sys.path:
    sys.path.insert(0, "/opt/trn_rl_repo")

import numpy as np

B_TOTAL = 32
B_PER_CORE = 4
N_CORES = 8
C = 512
N = 784  # 28*28, both spatial dims
FACTOR = 0.001

# n (and m) tiling: 6 tiles of 128 + one of 16
NT = [(0, 128), (128, 128), (256, 128), (384, 128), (512, 128), (640, 128), (768, 16)]
# free-dim split of 784 into PSUM-bank-sized pieces
HALVES = [(0, 512), (512, 272)]

_BUILT = {}


def _build(nbatch, enable_asserts=False):
    key = (nbatch, enable_asserts)
    if key in _BUILT:
        return _BUILT[key]

    import concourse.bass as bass
    import concourse.tile as tile
    from concourse import bacc, mybir
    from concourse.masks import make_identity

    f32 = mybir.dt.float32
    f32r = mybir.dt.float32r
    AF = mybir.ActivationFunctionType
    ALU = mybir.AluOpType

    nc = bacc.Bacc("TRN2", target_bir_lowering=False, debug=False,
                   enable_asserts=enable_asserts, num_devices=N_CORES)
    fm1 = nc.dram_tensor("fm1", [nbatch, C, N], f32, kind="ExternalInput").ap()
    fm2 = nc.dram_tensor("fm2", [nbatch, C, N], f32, kind="ExternalInput").ap()
    o1 = nc.dram_tensor("o1", [nbatch, C, N], f32, kind="ExternalOutput").ap()
    o2 = nc.dram_tensor("o2", [nbatch, C, N], f32, kind="ExternalOutput").ap()

    with tile.TileContext(nc) as tc:
        with (
            tc.tile_pool(name="sb", bufs=2) as sb,
            tc.tile_pool(name="ps", bufs=3, space="PSUM") as ps,
            tc.tile_pool(name="dr", bufs=2, space="DRAM") as dram,
        ):
            ident = sb.tile([128, 128], f32, tag="ident", bufs=1)
            make_identity(nc, ident[:])
            onesf = sb.tile([128, 1], f32, tag="onesf", bufs=1)
            nc.vector.memset(onesf[:], 1.0)
            ones = sb.tile([128, 1], f32r, tag="ones", bufs=1)
            nc.scalar.copy(ones[:], onesf[:])

            def bcast_row(row_ap, dram_tag, sb_tag):
                """Broadcast a [1, N] SBUF row to a [128, N] SBUF tile via a
                DRAM round-trip (DMA replicates with a 0-stride partition dim)."""
                d = dram.tile([1, N], f32, tag=dram_tag, bufs=2)
                nc.sync.dma_start(out=d[:], in_=row_ap)
                dap = d[:]
                src = bass.AP(tensor=dap.tensor, offset=dap.offset,
                              ap=[[0, 128]] + list(dap.ap))
                out = sb.tile([128, N], f32, tag=sb_tag, bufs=2)
                nc.gpsimd.dma_start(
                    out=out[:].rearrange("p (a x) -> p a x", a=1), in_=src)
                return out

            def transpose_and_norms(f_sb, ft_tag, negate):
                """PE-transpose f [c,n]->fT [n,c] (x FACTOR), and compute
                per-partition inverse column norms r = 1/||f_:i|| (optionally
                also -r in an f32r tile for feeding transposes)."""
                fT = sb.tile([128, 7 * 512], f32r, tag=ft_tag, bufs=1)
                ssq = sb.tile([128, 8], f32, tag=ft_tag + "_ssq", bufs=2)
                lg = sb.tile([128, 8], f32, tag=ft_tag + "_lg", bufs=2)
                r = sb.tile([128, 8], f32, tag=ft_tag + "_r", bufs=2)
                nr = None
                if negate:
                    nr = sb.tile([128, 8], f32, tag=ft_tag + "_nr", bufs=2)
                for t, (noff, nsz) in enumerate(NT):
                    pst = ps.tile([128, 512], f32, tag="pst", bufs=2)
                    for j in range(4):
                        nc.tensor.transpose(
                            pst[:nsz, j * 128:(j + 1) * 128],
                            f_sb[:, j, noff:noff + nsz].bitcast(f32),
                            ident[:, :])
                    fsl = fT[:nsz, t * 512:(t + 1) * 512]
                    nc.scalar.mul(fsl, pst[:nsz, :], FACTOR)
                    junk = sb.tile([128, 512], f32, tag="junk", bufs=2)
                    # ssq = sum_c (FACTOR*f)^2 via ACT Square w/ accumulate
                    nc.scalar.activation(
                        out=junk[:nsz], in_=fsl.bitcast(f32), func=AF.Square,
                        accum_out=ssq[:nsz, t:t + 1])
                    # r = 1/||f|| = exp(-0.5*ln(ssq*1e6))  (1e6 undoes FACTOR^2)
                    nc.scalar.activation(
                        out=lg[:nsz, t:t + 1], in_=ssq[:nsz, t:t + 1],
                        func=AF.Ln, scale=1e6)
                    nc.scalar.activation(
                        out=r[:nsz, t:t + 1], in_=lg[:nsz, t:t + 1],
                        func=AF.Exp, scale=-0.5)
                    if negate:
                        nc.vector.tensor_scalar_mul(
                            nr[:nsz, t:t + 1], r[:nsz, t:t + 1], -1.0)
                return fT, r, nr

            for b in range(nbatch):
                # ---- load f1, f2 as [128, 4(c-chunk), 784] f32r ----
                f1_sb = sb.tile([128, 4, N], f32r, tag="f1", bufs=2)
                nc.sync.dma_start(
                    out=f1_sb[:],
                    in_=fm1[b].rearrange("(t p) n -> p t n", p=128).bitcast(f32r))
                f2_sb = sb.tile([128, 4, N], f32r, tag="f2", bufs=2)
                nc.sync.dma_start(
                    out=f2_sb[:],
                    in_=fm2[b].rearrange("(t p) n -> p t n", p=128).bitcast(f32r))

                # ---- transposes + inverse norms ----
                f1T, r1c, _ = transpose_and_norms(f1_sb, "f1T", negate=False)
                f2T, _, nr2c = transpose_and_norms(f2_sb, "f2T", negate=True)

                # ---- -r2 as a [1, N] row -> broadcast to [128, N] ----
                prow = ps.tile([1, N], f32, tag="ps784", bufs=3)
                for t, (moff, msz) in enumerate(NT):
                    nc.tensor.transpose(
                        prow[:1, moff:moff + msz],
                        nr2c[:msz, t:t + 1],
                        ident[:msz, :msz])
                r2row = sb.tile([1, N], f32, tag="r2row", bufs=1)
                nc.scalar.copy(r2row[:1, :], prow[:1, :])
                negr2B = bcast_row(r2row[:1, :], "r2d", "negr2B")

                # ---- gram + exp (rowsum fused) + M2 = E/rowsum ----
                rsc = sb.tile([128, 8], f32, tag="rs", bufs=2)
                rrc = sb.tile([128, 8], f32, tag="rr", bufs=2)
                E = sb.tile([128, 7 * N], f32r, tag="E", bufs=1)
                M2s = []
                for t, (noff, nsz) in enumerate(NT):
                    G = ps.tile([128, N], f32, tag="ps784", bufs=3)
                    for j in range(4):
                        for (hoff, hsz) in HALVES:
                            nc.tensor.matmul(
                                G[:nsz, hoff:hoff + hsz],
                                f1_sb[:, j, noff:noff + nsz],
                                f2_sb[:, j, hoff:hoff + hsz],
                                start=(j == 0), stop=(j == 3))
                    Gs = sb.tile([128, N], f32, tag="gs", bufs=3)
                    nc.vector.tensor_mul(Gs[:nsz], G[:nsz], negr2B[:nsz])
                    Et = E[:nsz, t * N:(t + 1) * N]
                    nc.scalar.activation(
                        out=Et, in_=Gs[:nsz], func=AF.Exp,
                        scale=r1c[:nsz, t:t + 1],
                        accum_out=rsc[:nsz, t:t + 1])
                    nc.vector.reciprocal(rrc[:nsz, t:t + 1], rsc[:nsz, t:t + 1])
                    M2t = sb.tile([128, N], f32, tag="m2", bufs=7)
                    nc.vector.tensor_scalar_mul(M2t[:nsz], Et.bitcast(f32),
                                                rrc[:nsz, t:t + 1])
                    M2s.append(M2t)

                # ---- colsum(E) via ones-matmul -> rc2 row -> broadcast ----
                cs = ps.tile([1, N], f32, tag="ps784", bufs=3)
                for t, (noff, nsz) in enumerate(NT):
                    for (hoff, hsz) in HALVES:
                        nc.tensor.matmul(
                            cs[:1, hoff:hoff + hsz],
                            ones[:nsz, :1],
                            E[:nsz, t * N + hoff:t * N + hoff + hsz],
                            start=(t == 0), stop=(t == 6))
                rc2row = sb.tile([1, N], f32, tag="rc2row", bufs=1)
                nc.vector.reciprocal(rc2row[:1, :], cs[:1, :])
                rc2B = bcast_row(rc2row[:1, :], "rc2d", "rc2B")

                # ---- M2^T via PE transpose ----
                M2T = sb.tile([128, 7 * N], f32r, tag="m2t", bufs=1)
                for t, (moff, msz) in enumerate(NT):
                    pm = ps.tile([128, N], f32, tag="ps784", bufs=3)
                    for u, (noff, nsz) in enumerate(NT):
                        nc.tensor.transpose(
                            pm[:msz, noff:noff + nsz],
                            M2s[u][:nsz, moff:moff + msz],
                            ident[:nsz, :nsz])
                    nc.scalar.copy(M2T[:msz, t * N:(t + 1) * N], pm[:msz, :])

                # ---- outputs ----
                for ci in range(4):
                    csl = slice(ci * 128, (ci + 1) * 128)
                    # new_fm1[c,n] = sum_m f2T[m,c]*M2T[m,n]  (0.001, 1/rowsum folded)
                    P1 = ps.tile([128, N], f32, tag="ps784", bufs=3)
                    for t, (moff, msz) in enumerate(NT):
                        for (hoff, hsz) in HALVES:
                            nc.tensor.matmul(
                                P1[:, hoff:hoff + hsz],
                                f2T[:msz, t * 512 + ci * 128:
                                    t * 512 + (ci + 1) * 128],
                                M2T[:msz, t * N + hoff:t * N + hoff + hsz],
                                start=(t == 0), stop=(t == 6))
                    O1sb = sb.tile([128, N], f32, tag="o", bufs=4)
                    nc.scalar.copy(O1sb[:], P1[:])
                    nc.sync.dma_start(out=o1[b, csl, :], in_=O1sb[:])

                    # new_fm2[c,m] = (sum_n f1T[n,c]*E[n,m]) * rc2[m]
                    P2 = ps.tile([128, N], f32, tag="ps784", bufs=3)
                    for t, (noff, nsz) in enumerate(NT):
                        for (hoff, hsz) in HALVES:
                            nc.tensor.matmul(
                                P2[:, hoff:hoff + hsz],
                                f1T[:nsz, t * 512 + ci * 128:
                                    t * 512 + (ci + 1) * 128],
                                E[:nsz, t * N + hoff:t * N + hoff + hsz],
                                start=(t == 0), stop=(t == 6))
                    O2sb = sb.tile([128, N], f32, tag="o", bufs=4)
                    nc.vector.tensor_mul(O2sb[:], P2[:], rc2B[:])
                    nc.sync.dma_start(out=o2[b, csl, :], in_=O2sb[:])

    nc.compile()
    _BUILT[key] = nc
    return nc


def _run(fm1, fm2, trace=False):
    from concourse.bass_utils import run_bass_kernel_spmd

    fm1 = np.ascontiguousarray(np.asarray(fm1, np.float32).reshape(B_TOTAL, C, N))
    fm2 = np.ascontiguousarray(np.asarray(fm2, np.float32).reshape(B_TOTAL, C, N))
    nc = _build(B_PER_CORE)
    f1s = fm1.reshape(N_CORES, B_PER_CORE, C, N)
    f2s = fm2.reshape(N_CORES, B_PER_CORE, C, N)
    in_maps = [
        {"fm1": np.ascontiguousarray(f1s[i]), "fm2": np.ascontiguousarray(f2s[i])}
        for i in range(N_CORES)
    ]
    res = run_bass_kernel_spmd(nc, in_maps, core_ids=list(range(N_CORES)),
                               trace=trace)
    out1 = np.concatenate([res.results[i]["o1"] for i in range(N_CORES)], axis=0)
    out2 = np.concatenate([res.results[i]["o2"] for i in range(N_CORES)], axis=0)
    out1 = out1.reshape(B_TOTAL, C, 28, 28).astype(np.float32)
    out2 = out2.reshape(B_TOTAL, C, 28, 28).astype(np.float32)
    return (out1, out2), res


def kernel(fm1, fm2):
    (out1, out2), _ = _run(fm1, fm2)
    return out1, out2



# revision 4
# speedup vs baseline: 1.8009x; 1.8009x over previous
# Trainium2 Bass kernel for nn_FDM_3899830304921 (feature-map cosine-sim
# dual-softmax transport), data-parallel over the batch dim on 8 NeuronCores.
#
# Math per batch (c=512, n=m=784):
#   r1[n] = 1/||f1_:n||, r2[m] = 1/||f2_:m||
#   E[n,m]   = exp(-r1[n] * sum_c f1[c,n] * (f2[c,m]*r2[m]))   (= exp(-cos))
#   new_fm2  = (f1 @ E) * (0.001/colsum(E))      (softmax over n folded in)
#   new_fm1  = (f2 @ E^T) * (0.001/rowsum(E))    (softmax over m folded in)
#
# Engine assignment (per batch):
#   PE:   gram (f1^T f2), colsum ones-matmul, P1/P2 output matmuls,
#         14 tiny column->row transposes. All matmul operands bf16.
#   DMA:  IO + all big transposes (fT1, fT2, E^T) via the XBAR
#         dma_start_transpose path (2-byte dtype, 16x128 tiles).
#   ACT:  f32->bf16 input conversion, exp (w/ fused rowsum accumulation),
#         sqrt, tiny row copies. Only Exp+Sqrt use act tables.
#   DVE:  fused square+reduce for norms (tensor_tensor_reduce),
#         reciprocals, f2 prescale, output postscale multiplies.
#   Pool: pad memsets + partition_broadcast of the three scale rows.
#
# n/m are padded 784->896 (7x128) only where the XBAR transpose needs
# multiples of 128; pad lanes are zeroed or provably never read.
import sys

if "/opt/trn_rl_repo" not in sys.path:
    sys.path.insert(0, "/opt/trn_rl_repo")

import numpy as np

B_TOTAL = 32
B_PER_CORE = 4
N_CORES = 8
C = 512
N = 784  # 28*28, both spatial dims
NPAD = 896  # 7*128, for XBAR dma transposes
FACTOR = 0.001

# n (and m) tiling: 6 tiles of 128 + one of 16
NT = [(0, 128), (128, 128), (256, 128), (384, 128), (512, 128), (640, 128), (768, 16)]
# free-dim split of 784 into PSUM-bank-sized pieces
HALVES = [(0, 512), (512, 272)]

_BUILT = {}


def _build(nbatch, enable_asserts=False):
    key = (nbatch, enable_asserts)
    if key in _BUILT:
        return _BUILT[key]

    import concourse.bass as bass
    import concourse.tile as tile
    from concourse import bacc, mybir
    from concourse.masks import make_identity

    f32 = mybir.dt.float32
    bf16 = mybir.dt.bfloat16
    AF = mybir.ActivationFunctionType
    ALU = mybir.AluOpType

    nc = bacc.Bacc("TRN2", target_bir_lowering=False, debug=False,
                   enable_asserts=enable_asserts, num_devices=N_CORES)
    fm1 = nc.dram_tensor("fm1", [nbatch, C, N], f32, kind="ExternalInput").ap()
    fm2 = nc.dram_tensor("fm2", [nbatch, C, N], f32, kind="ExternalInput").ap()
    o1 = nc.dram_tensor("o1", [nbatch, C, N], f32, kind="ExternalOutput").ap()
    o2 = nc.dram_tensor("o2", [nbatch, C, N], f32, kind="ExternalOutput").ap()

    with tile.TileContext(nc) as tc:
        with (
            tc.tile_pool(name="sb", bufs=2) as sb,
            tc.tile_pool(name="ps", bufs=4, space="PSUM") as ps,
        ):
            ident = sb.tile([128, 128], f32, tag="ident", bufs=1)
            make_identity(nc, ident[:])
            onesb = sb.tile([128, 1], bf16, tag="onesb", bufs=1)
            nc.vector.memset(onesb[:], 1.0)

            def row_to_bcast(row_ps, tagstem, scale_to_recip):
                """[1,N] PSUM row -> (FACTOR/row) broadcast to [128,N] bf16.
                scale_to_recip: multiply row by 1/FACTOR before reciprocal
                so the broadcast tile is FACTOR/row."""
                x = sb.tile([1, N], f32, tag="rowx", bufs=4)
                nc.scalar.mul(x[:1, :], row_ps, scale_to_recip)
                rcp = sb.tile([1, N], f32, tag="rowr", bufs=4)
                nc.vector.reciprocal(rcp[:1, :], x[:1, :])
                rb = sb.tile([1, N], bf16, tag="rowb", bufs=4)
                nc.scalar.copy(rb[:1, :], rcp[:1, :])
                out = sb.tile([128, N], bf16, tag=tagstem + "B", bufs=2)
                nc.gpsimd.partition_broadcast(out[:], rb[:1, :])
                return out

            for b in range(nbatch):
                # ---- load raw f32 inputs ----
                fA = sb.tile([128, 4, N], f32, tag="fA", bufs=2)
                nc.sync.dma_start(
                    out=fA[:], in_=fm1[b].rearrange("(t p) n -> p t n", p=128))
                fB = sb.tile([128, 4, N], f32, tag="fB", bufs=2)
                nc.sync.dma_start(
                    out=fB[:], in_=fm2[b].rearrange("(t p) n -> p t n", p=128))

                # ---- convert to bf16 (padded to 896 for XBAR transposes) ----
                f1w = sb.tile([128, 4, NPAD], bf16, tag="f1w", bufs=2)
                f2w = sb.tile([128, 4, NPAD], bf16, tag="f2w", bufs=2)
                nc.gpsimd.memset(f1w[:, :, N:], 0.0)
                nc.gpsimd.memset(f2w[:, :, N:], 0.0)
                nc.scalar.copy(f1w[:, :, :N], fA[:])
                nc.scalar.copy(f2w[:, :, :N], fB[:])

                # ---- fT[p, t, c] = f[c, 128t+p] via XBAR dma transpose ----
                fT1 = sb.tile([128, 7, C], bf16, tag="fT1", bufs=2)
                fT2 = sb.tile([128, 7, C], bf16, tag="fT2", bufs=2)
                for j in range(4):
                    nc.sync.dma_start_transpose(
                        fT1[:, :, j * 128:(j + 1) * 128], f1w[:, j, :])
                    nc.sync.dma_start_transpose(
                        fT2[:, :, j * 128:(j + 1) * 128], f2w[:, j, :])

                # ---- norms: ssq = sum_c f^2 per spatial col (fused sq+reduce) ----
                ssq = sb.tile([128, 16], f32, tag="ssq", bufs=2)
                nc.vector.memset(ssq[:], 1.0)  # keep pad lanes finite
                junk = sb.tile([128, C], bf16, tag="junk", bufs=2)
                for t, (noff, nsz) in enumerate(NT):
                    nc.vector.tensor_mul(junk[:nsz], fT1[:nsz, t, :],
                                         fT1[:nsz, t, :])
                    nc.vector.reduce_sum(ssq[:nsz, t:t + 1], junk[:nsz],
                                         axis=mybir.AxisListType.X)
                    nc.vector.tensor_mul(junk[:nsz], fT2[:nsz, t, :],
                                         fT2[:nsz, t, :])
                    nc.vector.reduce_sum(ssq[:nsz, 8 + t:9 + t], junk[:nsz],
                                         axis=mybir.AxisListType.X)
                s = sb.tile([128, 16], f32, tag="s", bufs=2)
                nc.scalar.sqrt(s[:], ssq[:])
                r = sb.tile([128, 16], f32, tag="r", bufs=2)
                nc.vector.reciprocal(r[:], s[:])
                r1n = sb.tile([128, 8], f32, tag="r1n", bufs=2)
                nc.vector.tensor_scalar_mul(r1n[:, :7], r[:, :7], -1.0)

                # ---- r2 cols -> [1,N] row -> bf16 -> broadcast [128,N] ----
                prow = ps.tile([1, N], f32, tag="big", bufs=4)
                for t, (moff, msz) in enumerate(NT):
                    nc.tensor.transpose(
                        prow[:1, moff:moff + msz], r[:msz, 8 + t:9 + t],
                        ident[:msz, :msz])
                r2row = sb.tile([1, N], bf16, tag="rowb", bufs=4)
                nc.scalar.copy(r2row[:1, :], prow[:1, :])
                r2B = sb.tile([128, N], bf16, tag="r2B", bufs=2)
                nc.gpsimd.partition_broadcast(r2B[:], r2row[:1, :])

                # ---- prescale: f2s[c,m] = f2[c,m] * r2[m]  (bf16) ----
                f2s = sb.tile([128, 4, N], bf16, tag="f2s", bufs=2)
                for j in range(4):
                    nc.vector.tensor_mul(f2s[:, j, :], f2w[:, j, :N], r2B[:])

                # ---- gram + exp (rowsum fused via accum) ----
                rsc = sb.tile([128, 8], f32, tag="rsc", bufs=2)
                E = sb.tile([128, 7, NPAD], bf16, tag="E", bufs=1)
                nc.gpsimd.memset(E[:, :, N:], 0.0)     # pad cols (XBAR reads)
                nc.gpsimd.memset(E[:, 6, :N], 0.0)     # rows 16+ of tile 6 stay 0
                for t, (noff, nsz) in enumerate(NT):
                    G = ps.tile([128, N], f32, tag="big", bufs=4)
                    for j in range(4):
                        for hoff, hsz in HALVES:
                            nc.tensor.matmul(
                                G[:nsz, hoff:hoff + hsz],
                                f1w[:, j, noff:noff + nsz],
                                f2s[:, j, hoff:hoff + hsz],
                                start=(j == 0), stop=(j == 3))
                    nc.scalar.activation(
                        out=E[:nsz, t, :N], in_=G[:nsz, :N], func=AF.Exp,
                        scale=r1n[:nsz, t:t + 1],
                        accum_out=rsc[:nsz, t:t + 1])

                # ---- ET[p, t, n] = E[n, 128t+p] via XBAR dma transpose ----
                ET = sb.tile([128, 7, NPAD], bf16, tag="ET", bufs=1)
                for u in range(7):
                    nc.sync.dma_start_transpose(
                        ET[:, :, u * 128:(u + 1) * 128], E[:, u, :])

                # ---- colsum(E) row via ones-matmul, then bcast of 0.001/cs ----
                csp = ps.tile([1, N], f32, tag="big", bufs=4)
                for t, (noff, nsz) in enumerate(NT):
                    for hoff, hsz in HALVES:
                        nc.tensor.matmul(
                            csp[:1, hoff:hoff + hsz], onesb[:nsz, :1],
                            E[:nsz, t, hoff:hoff + hsz],
                            start=(t == 0), stop=(t == 6))
                rcB = row_to_bcast(csp[:1, :N], "rc", 1.0 / FACTOR)

                # ---- rowsum cols -> row -> bcast of 0.001/rs ----
                rsp = ps.tile([1, N], f32, tag="big", bufs=4)
                for t, (noff, nsz) in enumerate(NT):
                    nc.tensor.transpose(
                        rsp[:1, noff:noff + nsz], rsc[:nsz, t:t + 1],
                        ident[:nsz, :nsz])
                rsB = row_to_bcast(rsp[:1, :N], "rs", 1.0 / FACTOR)

                # ---- new_fm2: P2[c,m] = sum_n f1[c,n] E[n,m]; O2 = P2*rcB ----
                for ci in range(4):
                    P = ps.tile([128, N], f32, tag="big", bufs=4)
                    for t, (noff, nsz) in enumerate(NT):
                        for hoff, hsz in HALVES:
                            nc.tensor.matmul(
                                P[:, hoff:hoff + hsz],
                                fT1[:nsz, t, ci * 128:(ci + 1) * 128],
                                E[:nsz, t, hoff:hoff + hsz],
                                start=(t == 0), stop=(t == 6))
                    O2 = sb.tile([128, N], f32, tag="O", bufs=4)
                    nc.vector.tensor_mul(O2[:], P[:], rcB[:])
                    nc.sync.dma_start(
                        out=o2[b, ci * 128:(ci + 1) * 128, :], in_=O2[:])

                # ---- new_fm1: P1[c,n] = sum_m f2[c,m] ET[m,n]; O1 = P1*rsB ----
                for ci in range(4):
                    P = ps.tile([128, N], f32, tag="big", bufs=4)
                    for t, (moff, msz) in enumerate(NT):
                        for hoff, hsz in HALVES:
                            nc.tensor.matmul(
                                P[:, hoff:hoff + hsz],
                                fT2[:msz, t, ci * 128:(ci + 1) * 128],
                                ET[:msz, t, hoff:hoff + hsz],
                                start=(t == 0), stop=(t == 6))
                    O1 = sb.tile([128, N], f32, tag="O", bufs=4)
                    nc.vector.tensor_mul(O1[:], P[:], rsB[:])
                    nc.sync.dma_start(
                        out=o1[b, ci * 128:(ci + 1) * 128, :], in_=O1[:])

    nc.compile()
    _BUILT[key] = nc
    return nc


def _run(fm1, fm2, trace=False):
    from concourse.bass_utils import run_bass_kernel_spmd

    fm1 = np.ascontiguousarray(np.asarray(fm1, np.float32).reshape(B_TOTAL, C, N))
    fm2 = np.ascontiguousarray(np.asarray(fm2, np.float32).reshape(B_TOTAL, C, N))
    nc = _build(B_PER_CORE)
    f1s = fm1.reshape(N_CORES, B_PER_CORE, C, N)
    f2s = fm2.reshape(N_CORES, B_PER_CORE, C, N)
    in_maps = [
        {"fm1": np.ascontiguousarray(f1s[i]), "fm2": np.ascontiguousarray(f2s[i])}
        for i in range(N_CORES)
    ]
    res = run_bass_kernel_spmd(nc, in_maps, core_ids=list(range(N_CORES)),
                               trace=trace)
    out1 = np.concatenate([res.results[i]["o1"] for i in range(N_CORES)], axis=0)
    out2 = np.concatenate([res.results[i]["o2"] for i in range(N_CORES)], axis=0)
    out1 = out1.reshape(B_TOTAL, C, 28, 28).astype(np.float32)
    out2 = out2.reshape(B_TOTAL, C, 28, 28).astype(np.float32)
    return (out1, out2), res


def kernel(fm1, fm2):
    (out1, out2), _ = _run(fm1, fm2)
    return out1, out2
